# revision 1
# baseline (speedup 1.0000x reference)
"""CapsuleLayer dynamic-routing kernel for TRN2, 8 NeuronCores, batch-sharded.

Per core: B_loc=8, I=2048, K=16, D=8, E=16.
SBUF layout: partitions p = i_sub*8 + b (16 i's per block x 8 batches), 128 j-blocks.
u_hat created via block-diagonal matmuls (stationary = blkdiag(inputs), moving = W),
routing sums via blkdiag-ones matmuls with PSUM accumulation; softmax/squash on DVE/ACT.
Host pre-packs all layouts (bf16 cast + transpose + blkdiag) in numpy.
"""
import sys
sys.path.insert(0, "/opt/trn_rl_repo")

import numpy as np
import ml_dtypes

import concourse.bass as bass
import concourse.tile as tile
from concourse import bacc, mybir
from concourse.bass_utils import run_bass_kernel_spmd

NCORES = 8
B, I, K, D, E = 64, 2048, 16, 8, 16
BL = B // NCORES          # 8 batches per core
NJ = I // 16              # 128 blocks of 16 input capsules
JC = 16                   # j-blocks per routing chunk
EPS = 1e-7

bf16 = mybir.dt.bfloat16
f32 = mybir.dt.float32
FT = mybir.ActivationFunctionType

TRACE = False
_NC_CACHE = {}


def _bc(ap, shape):
    try:
        return ap.broadcast_to(shape)
    except Exception:
        return ap.to_broadcast(shape)


def _capsule_kernel(tc, vout, ablk, wmv, onesa, onesb):
    nc = tc.nc
    with (
        tc.tile_pool(name="singles", bufs=1) as singles,
        tc.tile_pool(name="wstream", bufs=6) as wpool,
        tc.tile_pool(name="crps", bufs=5, space="PSUM") as crps,
        tc.tile_pool(name="sps", bufs=2, space="PSUM") as sps,
        tc.tile_pool(name="chunk", bufs=3) as chpool,
        tc.tile_pool(name="small", bufs=3) as small,
        tc.tile_pool(name="vreps", bufs=2) as vreps,
    ):
        ones_a = singles.tile([128, 8], bf16)
        nc.sync.dma_start(out=ones_a, in_=onesa)
        ones_b = singles.tile([128, 8], bf16)
        nc.sync.dma_start(out=ones_b, in_=onesb)
        ablk_sb = singles.tile([128, NJ, 128], bf16)
        nc.sync.dma_start(out=ablk_sb, in_=ablk)

        u_bf = singles.tile([128, NJ, K, E], bf16)      # 8 MiB
        logits = singles.tile([128, NJ, K], f32)        # 1 MiB

        # ---- phase 1: u_hat creation + s0 = (1/16) sum_i u_hat ----
        s_ps = sps.tile([8, K, E], f32)
        for j in range(NJ):
            wt = wpool.tile([128, 256], bf16)
            nc.sync.dma_start(out=wt, in_=wmv[j])
            ps = crps.tile([128, K, E], f32)
            nc.tensor.matmul(ps, lhsT=ablk_sb[:, j], rhs=wt,
                             start=True, stop=True, skip_group_check=True)
            if j % 2 == 0:
                nc.vector.tensor_copy(u_bf[:, j], ps)
            else:
                nc.scalar.activation(u_bf[:, j], ps, func=FT.Copy)
            nc.tensor.matmul(s_ps, lhsT=ones_a, rhs=u_bf[:, j],
                             start=(j == 0), stop=(j == NJ - 1),
                             skip_group_check=True)

        def squash(s_psum, make_rep):
            s_sb = small.tile([8, K, E], f32, tag="s_sb")
            nc.vector.tensor_copy(s_sb, s_psum)
            sq = small.tile([8, K, E], f32, tag="sq")
            nc.vector.tensor_mul(sq, s_sb, s_sb)
            t8 = small.tile([8, K, 8], f32, tag="sq8")
            nc.vector.tensor_add(t8, sq[:, :, 0:8], sq[:, :, 8:16])
            t4 = small.tile([8, K, 4], f32, tag="sq4")
            nc.vector.tensor_add(t4, t8[:, :, 0:4], t8[:, :, 4:8])
            t2 = small.tile([8, K, 2], f32, tag="sq2")
            nc.vector.tensor_add(t2, t4[:, :, 0:2], t4[:, :, 2:4])
            sn = small.tile([8, K], f32, tag="sn")
            nc.vector.tensor_add(sn, t2[:, :, 0], t2[:, :, 1])
            sne = small.tile([8, K], f32, tag="sne")
            nc.vector.tensor_scalar_add(sne, sn, EPS)
            sqr = small.tile([8, K], f32, tag="sqr")
            nc.scalar.activation(sqr, sne, func=FT.Sqrt)
            onep = small.tile([8, K], f32, tag="onep")
            nc.vector.tensor_scalar_add(onep, sn, 1.0)
            den = small.tile([8, K], f32, tag="den")
            nc.vector.tensor_mul(den, sqr, onep)
            rec = small.tile([8, K], f32, tag="recd")
            nc.vector.reciprocal(rec, den)
            fac = small.tile([8, K], f32, tag="fac")
            nc.vector.tensor_mul(fac, sn, rec)
            v_sb = small.tile([8, K, E], f32, tag="v_sb")
            nc.vector.tensor_mul(v_sb, s_sb, _bc(fac.unsqueeze(2), [8, K, E]))
            if not make_rep:
                return v_sb, None
            v_rep = vreps.tile([128, K, E], bf16, tag="v_rep")
            nc.vector.tensor_copy(v_rep[0:8], v_sb)
            for g in range(1, 16):
                nc.sync.dma_start(out=v_rep[8 * g:8 * g + 8], in_=v_rep[0:8])
            return v_sb, v_rep

        _, v_rep = squash(s_ps, True)

        # ---- routing iterations ----
        v_final = None
        for r in (1, 2):
            s_ps = sps.tile([8, K, E], f32)
            for ci in range(NJ // JC):
                jsl = slice(ci * JC, (ci + 1) * JC)
                # agreement: logits[:, jsl, k] (+)= sum_e u*v
                prod = chpool.tile([128, JC, K, E], bf16, tag="prod")
                peng = nc.gpsimd if ci % 2 == 0 else nc.vector
                peng.tensor_mul(
                    prod, u_bf[:, jsl],
                    _bc(v_rep.unsqueeze(1), [128, JC, K, E]))
                a8 = chpool.tile([128, JC, K, 8], bf16, tag="a8")
                nc.vector.tensor_add(a8, prod[:, :, :, 0:8], prod[:, :, :, 8:16])
                a4 = chpool.tile([128, JC, K, 4], bf16, tag="a4")
                nc.vector.tensor_add(a4, a8[:, :, :, 0:4], a8[:, :, :, 4:8])
                a2 = chpool.tile([128, JC, K, 2], bf16, tag="a2")
                nc.vector.tensor_add(a2, a4[:, :, :, 0:2], a4[:, :, :, 2:4])
                if r == 1:
                    nc.vector.tensor_add(logits[:, jsl], a2[:, :, :, 0], a2[:, :, :, 1])
                else:
                    a1 = chpool.tile([128, JC, K], f32, tag="a1")
                    nc.vector.tensor_add(a1, a2[:, :, :, 0], a2[:, :, :, 1])
                    nc.vector.tensor_add(logits[:, jsl], logits[:, jsl], a1)
                # softmax over k
                ex = chpool.tile([128, JC, K], f32, tag="ex")
                nc.scalar.activation(ex, logits[:, jsl], func=FT.Exp)
                k8 = chpool.tile([128, JC, 8], f32, tag="k8")
                nc.vector.tensor_add(k8, ex[:, :, 0:8], ex[:, :, 8:16])
                k4 = chpool.tile([128, JC, 4], f32, tag="k4")
                nc.vector.tensor_add(k4, k8[:, :, 0:4], k8[:, :, 4:8])
                k2 = chpool.tile([128, JC, 2], f32, tag="k2")
                nc.vector.tensor_add(k2, k4[:, :, 0:2], k4[:, :, 2:4])
                ks = chpool.tile([128, JC], f32, tag="ks")
                nc.vector.tensor_add(ks, k2[:, :, 0], k2[:, :, 1])
                krec = chpool.tile([128, JC], f32, tag="krec")
                nc.vector.reciprocal(krec, ks)
                cch = chpool.tile([128, JC, K], bf16, tag="cch")
                nc.vector.tensor_mul(cch, ex, _bc(krec.unsqueeze(2), [128, JC, K]))
                cu = chpool.tile([128, JC, K, E], bf16, tag="cu")
                cueng = nc.vector if ci % 2 == 0 else nc.gpsimd
                cueng.tensor_mul(cu, u_bf[:, jsl],
                                 _bc(cch.unsqueeze(3), [128, JC, K, E]))
                for jj in range(JC):
                    nc.tensor.matmul(
                        s_ps, lhsT=ones_b, rhs=cu[:, jj],
                        start=(ci == 0 and jj == 0),
                        stop=(ci == NJ // JC - 1 and jj == JC - 1),
                        skip_group_check=True)
            v_sb, v_rep = squash(s_ps, r != 2)
            v_final = v_sb

        nc.sync.dma_start(out=vout, in_=v_final)


def _build():
    if "nc" in _NC_CACHE:
        return _NC_CACHE["nc"]
    nc = bacc.Bacc("TRN2", target_bir_lowering=False, debug=False,
                   num_devices=NCORES)
    ablk = nc.dram_tensor("ablk", [128, NJ, 128], bf16, kind="ExternalInput").ap()
    wmv = nc.dram_tensor("wmv", [NJ, 128, 256], bf16, kind="ExternalInput").ap()
    onesa = nc.dram_tensor("onesa", [128, 8], bf16, kind="ExternalInput").ap()
    onesb = nc.dram_tensor("onesb", [128, 8], bf16, kind="ExternalInput").ap()
    vout = nc.dram_tensor("vout", [BL, K, E], f32, kind="ExternalOutput").ap()
    with tile.TileContext(nc) as tc:
        _capsule_kernel(tc, vout, ablk, wmv, onesa, onesb)
    nc.compile()
    _NC_CACHE["nc"] = nc
    return nc


def kernel(inputs, W):
    inputs = np.asarray(inputs, np.float32)
    W = np.asarray(W, np.float32)
    nc = _build()

    # W[i,k,d,e] -> [j, (i16 d), (k e)] bf16, contiguous per block
    Wb = np.ascontiguousarray(
        W.reshape(NJ, 16, K, D, E).transpose(0, 1, 3, 2, 4)
    ).reshape(NJ, 128, 256).astype(ml_dtypes.bfloat16)

    onesa_np = np.zeros((128, 8), np.float32)
    onesa_np[np.arange(128), np.arange(128) % 8] = 1.0 / 16.0
    onesb_np = (onesa_np * 16.0).astype(ml_dtypes.bfloat16)
    onesa_np = onesa_np.astype(ml_dtypes.bfloat16)

    in_maps = []
    for c in range(NCORES):
        inp_c = inputs[c * BL:(c + 1) * BL]          # [8, 2048, 8]
        inp_t = inp_c.reshape(BL, NJ, 16, D)          # b, j, iu, d
        ab = np.zeros((16, D, NJ, 16, BL), np.float32)  # iu d j iu2 b
        for iu in range(16):
            ab[iu, :, :, iu, :] = inp_t[:, :, iu, :].transpose(2, 1, 0)
        ab = ab.reshape(128, NJ, 128).astype(ml_dtypes.bfloat16)
        in_maps.append({"ablk": ab, "wmv": Wb,
                        "onesa": onesa_np, "onesb": onesb_np})

    br = run_bass_kernel_spmd(nc, in_maps, core_ids=list(range(NCORES)),
                              trace=TRACE)
    if br.exec_time_ns is not None:
        print(f"HW exec time: {br.exec_time_ns} ns")
    out = np.concatenate([r["vout"] for r in br.results], axis=0)
    return out.astype(np.float32)



# revision 5
# speedup vs baseline: 1.5205x; 1.5205x over previous
"""CapsuleLayer dynamic-routing kernel for TRN2, 8 NeuronCores, batch-sharded.

Per core: B_loc=8, I=2048, K=16, D=8, E=16.
Partitions p = b*16 + iu (8 batches x 16 input-capsules per j-block), NJ=128 j-blocks.
u_hat stored [p, j, e, k] bf16 (k packed last so every big DVE op hits 2x mode).

Phase 1: u_hat via block-diagonal matmuls (lhsT = blkdiag(inputs), rhs = W tile),
W streamed in 8 batched DMAs; s0 accumulated with a ones/16 lhsT matmul chain.
Routing: agreement u.v via one DVE mul + e-reduction tree (all bf16, 2x mode);
softmax over k on DVE/ACT; coupling coefficients scattered into a block-diagonal
C matrix (4x-mode copies) used as matmul lhsT so the weighted sum s = sum_i c*u
runs entirely on the PE with f32 PSUM accumulation; squash is sqrt-free
(rsqrt via exp(-0.5*ln)) keeping ACT on one function table.
"""
import sys
sys.path.insert(0, "/opt/trn_rl_repo")

import numpy as np
import ml_dtypes

import concourse.bass as bass
import concourse.tile as tile
from concourse import bacc, mybir
from concourse.bass_utils import run_bass_kernel_spmd

NCORES = 8
B, I, K, D, E = 64, 2048, 16, 8, 16
BL = B // NCORES          # 8 batches per core
NJ = I // 16              # 128 blocks of 16 input capsules
JC = 32                   # j-blocks per routing chunk
NCH = NJ // JC            # 4 chunks
WCH = 16                  # j-blocks per W DMA chunk
EPS = 1e-7

bf16 = mybir.dt.bfloat16
f32 = mybir.dt.float32
FT = mybir.ActivationFunctionType

TRACE = False
_NC_CACHE = {}


def _bc(ap, shape):
    try:
        return ap.broadcast_to(shape)
    except Exception:
        return ap.to_broadcast(shape)


def _capsule_kernel(tc, vout, ablk, wmv, onesa, repmat):
    nc = tc.nc
    with (
        tc.tile_pool(name="singles", bufs=1) as singles,
        tc.tile_pool(name="wstream", bufs=2) as wpool,
        tc.tile_pool(name="crps", bufs=2, space="PSUM") as crps,
        tc.tile_pool(name="sps", bufs=1, space="PSUM") as sps,
        tc.tile_pool(name="saccps", bufs=1, space="PSUM") as saccps,
        tc.tile_pool(name="vrps", bufs=1, space="PSUM") as vrps,
        tc.tile_pool(name="bigchunk", bufs=1) as bchp,
        tc.tile_pool(name="chunk", bufs=2) as chp,
        tc.tile_pool(name="small", bufs=2) as small,
    ):
        ones_a = singles.tile([128, 8], bf16)
        nc.sync.dma_start(out=ones_a, in_=onesa)
        repm = singles.tile([8, 128], bf16)
        nc.sync.dma_start(out=repm, in_=repmat)

        u_bf = singles.tile([128, NJ, E, K], bf16)      # 8 MiB, layout (j, e, k)
        a_r1 = singles.tile([128, NJ, K], bf16)         # agreement logits A(v0)
        L = singles.tile([128, NJ, 8, K], bf16)         # blockdiag C, (j, b', k)
        nc.gpsimd.memset(L, 0.0)                        # zeros persist; only diag rewritten

        ablk_sb = singles.tile([128, NJ, 128], bf16)
        for c in range(4):
            nc.sync.dma_start(out=ablk_sb[:, 32 * c:32 * (c + 1)],
                              in_=ablk[:, 32 * c:32 * (c + 1)])

        # ---- phase 1: u_hat + s0 = (1/16) sum_i u_hat ----
        s0_ps = sps.tile([8, E, K], f32)
        g_idx = 0
        for c in range(NJ // WCH):
            wt = wpool.tile([128, WCH, 256], bf16)
            nc.sync.dma_start(out=wt, in_=wmv[:, c * WCH:(c + 1) * WCH])
            for g in range(WCH // 4):
                j0 = c * WCH + g * 4
                ps = crps.tile([128, 4, 256], f32)
                for jj in range(4):
                    nc.tensor.matmul(ps[:, jj], lhsT=ablk_sb[:, j0 + jj],
                                     rhs=wt[:, g * 4 + jj],
                                     start=True, stop=True, skip_group_check=True)
                dst = u_bf[:, j0:j0 + 4]
                m = g_idx % 4
                g_idx += 1
                if m in (0, 1):
                    nc.scalar.activation(dst, ps, func=FT.Copy)
                elif m == 2:
                    nc.vector.tensor_copy(dst, ps)
                else:
                    nc.gpsimd.tensor_copy(dst, ps)
                for jj in range(4):
                    j = j0 + jj
                    nc.tensor.matmul(s0_ps, lhsT=ones_a, rhs=u_bf[:, j],
                                     start=(j == 0), stop=(j == NJ - 1),
                                     skip_group_check=True)

        def squash(sdiag, final):
            # sdiag [8, E, K] f32; returns v_rep [128, E, K] bf16 (unless final)
            sq = small.tile([8, E, K], f32, tag="sq")
            nc.vector.tensor_mul(sq, sdiag, sdiag)
            t8 = small.tile([8, 8, K], f32, tag="sq8")
            nc.vector.tensor_add(t8, sq[:, 0:8], sq[:, 8:16])
            t4 = small.tile([8, 4, K], f32, tag="sq4")
            nc.vector.tensor_add(t4, t8[:, 0:4], t8[:, 4:8])
            t2 = small.tile([8, 2, K], f32, tag="sq2")
            nc.vector.tensor_add(t2, t4[:, 0:2], t4[:, 2:4])
            sn = small.tile([8, K], f32, tag="sn")
            nc.vector.tensor_add(sn, t2[:, 0], t2[:, 1])
            sne = small.tile([8, K], f32, tag="sne")
            nc.vector.tensor_scalar_add(sne, sn, EPS)
            lns = small.tile([8, K], f32, tag="lns")
            nc.scalar.activation(lns, sne, func=FT.Ln)
            rst = small.tile([8, K], f32, tag="rst")      # rsqrt(sn+eps)
            nc.scalar.activation(rst, lns, func=FT.Exp, scale=-0.5)
            onep = small.tile([8, K], f32, tag="onep")
            nc.vector.tensor_scalar_add(onep, sn, 1.0)
            rec = small.tile([8, K], f32, tag="rec")
            nc.vector.reciprocal(rec, onep)
            fac = small.tile([8, K], f32, tag="fac")
            nc.vector.tensor_mul(fac, sn, rst)
            fac2 = small.tile([8, K], f32, tag="fac2")
            nc.vector.tensor_mul(fac2, fac, rec)
            if final:
                vfin = small.tile([8, K, E], f32, tag="vfin")
                # vfin[b,k,e] = sdiag[b,e,k] * fac2[b,k]; iterate (e,k) strided out
                nc.vector.tensor_mul(
                    vfin.rearrange("b k e -> b e k"), sdiag,
                    _bc(fac2.unsqueeze(1), [8, E, K]))
                return vfin
            vbf = small.tile([8, E, K], bf16, tag="vbf")
            nc.vector.tensor_mul(vbf, sdiag,
                                 _bc(fac2.unsqueeze(1), [8, E, K]))
            vr_ps = vrps.tile([128, E, K], f32)
            nc.tensor.matmul(vr_ps, lhsT=repm, rhs=vbf,
                             start=True, stop=True, skip_group_check=True)
            v_rep = small.tile([128, E, K], bf16, tag="vrep")
            nc.vector.tensor_copy(v_rep, vr_ps)
            return v_rep

        s0_sb = small.tile([8, E, K], f32, tag="s0sb")
        nc.vector.tensor_copy(s0_sb, s0_ps)
        v_rep = squash(s0_sb, False)

        # ---- routing iterations ----
        for r in (1, 2):
            s_ps = saccps.tile([128, E, K], f32)
            for ci in range(NCH):
                jsl = slice(ci * JC, (ci + 1) * JC)
                prod = bchp.tile([128, JC, E, K], bf16, tag="prod")
                nc.vector.tensor_mul(prod, u_bf[:, jsl],
                                     _bc(v_rep.unsqueeze(1), [128, JC, E, K]))
                a8 = bchp.tile([128, JC, 8, K], bf16, tag="a8")
                nc.vector.tensor_add(a8, prod[:, :, 0:8], prod[:, :, 8:16])
                a4 = bchp.tile([128, JC, 4, K], bf16, tag="a4")
                nc.vector.tensor_add(a4, a8[:, :, 0:4], a8[:, :, 4:8])
                a2 = bchp.tile([128, JC, 2, K], bf16, tag="a2")
                nc.vector.tensor_add(a2, a4[:, :, 0:2], a4[:, :, 2:4])
                if r == 1:
                    nc.vector.tensor_add(a_r1[:, jsl], a2[:, :, 0], a2[:, :, 1])
                    ex_src = a_r1[:, jsl]
                else:
                    acomb = chp.tile([128, JC, K], bf16, tag="acomb")
                    nc.vector.tensor_add(acomb, a2[:, :, 0], a2[:, :, 1])
                    nc.vector.tensor_add(acomb, acomb, a_r1[:, jsl])
                    ex_src = acomb
                ex = chp.tile([128, JC, K], bf16, tag="ex")
                nc.scalar.activation(ex, ex_src, func=FT.Exp)
                k8 = chp.tile([128, JC, 8], bf16, tag="k8")
                nc.vector.tensor_add(k8, ex[:, :, 0:8], ex[:, :, 8:16])
                k4 = chp.tile([128, JC, 4], bf16, tag="k4")
                nc.vector.tensor_add(k4, k8[:, :, 0:4], k8[:, :, 4:8])
                k2 = chp.tile([128, JC, 2], bf16, tag="k2")
                nc.vector.tensor_add(k2, k4[:, :, 0:2], k4[:, :, 2:4])
                ks = chp.tile([128, JC], f32, tag="ks")
                nc.vector.tensor_add(ks, k2[:, :, 0], k2[:, :, 1])
                krec = chp.tile([128, JC], f32, tag="krec")
                nc.vector.reciprocal(krec, ks)
                cch = chp.tile([128, JC, K], bf16, tag="cch")
                nc.vector.tensor_mul(cch, ex, _bc(krec.unsqueeze(2), [128, JC, K]))
                for b in range(8):
                    nc.vector.tensor_copy(L[b * 16:(b + 1) * 16, jsl, b, :],
                                          cch[b * 16:(b + 1) * 16])
                for jj in range(JC):
                    j = ci * JC + jj
                    nc.tensor.matmul(s_ps, lhsT=L[:, j], rhs=u_bf[:, j],
                                     start=(j == 0), stop=(j == NJ - 1),
                                     skip_group_check=True)
            sdiag = small.tile([8, E, K], f32, tag="sdiag")
            for k in range(K):
                nc.gpsimd.tensor_copy(sdiag[:, :, k], s_ps[k::16, :, k])
            if r == 2:
                vfin = squash(sdiag, True)
                nc.sync.dma_start(out=vout, in_=vfin)
            else:
                v_rep = squash(sdiag, False)


def _build():
    if "nc" in _NC_CACHE:
        return _NC_CACHE["nc"]
    nc = bacc.Bacc("TRN2", target_bir_lowering=False, debug=False,
                   num_devices=NCORES)
    ablk = nc.dram_tensor("ablk", [128, NJ, 128], bf16, kind="ExternalInput").ap()
    wmv = nc.dram_tensor("wmv", [128, NJ, 256], bf16, kind="ExternalInput").ap()
    onesa = nc.dram_tensor("onesa", [128, 8], bf16, kind="ExternalInput").ap()
    repmat = nc.dram_tensor("repmat", [8, 128], bf16, kind="ExternalInput").ap()
    vout = nc.dram_tensor("vout", [BL, K, E], f32, kind="ExternalOutput").ap()
    with tile.TileContext(nc) as tc:
        _capsule_kernel(tc, vout, ablk, wmv, onesa, repmat)
    nc.compile()
    _NC_CACHE["nc"] = nc
    return nc


def kernel(inputs, W):
    inputs = np.asarray(inputs, np.float32)
    W = np.asarray(W, np.float32)
    nc = _build()

    # W[i,k,d,e] -> [j, iu, d, e, k] -> [(iu d)=128, j, (e k)=256] bf16
    Wb = np.ascontiguousarray(
        W.reshape(NJ, 16, K, D, E).transpose(0, 1, 3, 4, 2)
        .reshape(NJ, 128, 256).transpose(1, 0, 2)
    ).astype(ml_dtypes.bfloat16)

    onesa_np = np.zeros((128, 8), np.float32)
    onesa_np[np.arange(128), np.arange(128) // 16] = 1.0 / 16.0
    onesa_np = onesa_np.astype(ml_dtypes.bfloat16)
    repmat_np = np.zeros((8, 128), np.float32)
    repmat_np[np.arange(128) // 16, np.arange(128)] = 1.0
    repmat_np = repmat_np.astype(ml_dtypes.bfloat16)

    in_maps = []
    for c in range(NCORES):
        inp_c = inputs[c * BL:(c + 1) * BL]               # [8, 2048, 8]
        inp_t = inp_c.reshape(BL, NJ, 16, D)              # b, j, iu, d
        ab = np.zeros((16, D, NJ, BL, 16), np.float32)    # iu d j b iu2
        for iu in range(16):
            ab[iu, :, :, :, iu] = inp_t[:, :, iu, :].transpose(2, 1, 0)
        ab = ab.reshape(128, NJ, 128).astype(ml_dtypes.bfloat16)
        in_maps.append({"ablk": ab, "wmv": Wb,
                        "onesa": onesa_np, "repmat": repmat_np})

    br = run_bass_kernel_spmd(nc, in_maps, core_ids=list(range(NCORES)),
                              trace=TRACE)
    if br.exec_time_ns is not None:
        print(f"HW exec time: {br.exec_time_ns} ns")
    out = np.concatenate([r["vout"] for r in br.results], axis=0)
    return out.astype(np.float32)


# revision 7
# speedup vs baseline: 1.6169x; 1.0634x over previous
"""CapsuleLayer dynamic-routing kernel for TRN2, 8 NeuronCores, batch-sharded.

Per core: B_loc=8, I=2048, K=16, D=8, E=16.
Partitions p = b*16 + iu (8 batches x 16 input-capsules per j-block), NJ=128 j-blocks.
u_hat stored [p, j, e, k] bf16 (k packed last so every big DVE op hits 2x mode).

Phase 1: u_hat via block-diagonal matmuls (lhsT = blkdiag(inputs), rhs = W tile),
W streamed in 8 batched DMAs; s0 accumulated straight off the W tiles with a
dense input-sum lhsT so the PE chain never waits on the PSUM->SBUF copies.
Routing: agreement u.v via one DVE mul + e-reduction tree (all bf16, 2x mode);
softmax over k; coupling coefficients scattered into a block-diagonal C matrix
(4x-mode copies, split DVE/Pool) used as matmul lhsT so the weighted sum
s = sum_i c*u runs on the PE with f32 PSUM accumulation. Squash is all-DVE
(Quake rsqrt + 2 Newton steps) so ACT only ever runs Copy/Exp (one table load).
"""
import sys
sys.path.insert(0, "/opt/trn_rl_repo")

import numpy as np
import ml_dtypes

import concourse.bass as bass
import concourse.tile as tile
from concourse import bacc, mybir
from concourse.bass_utils import run_bass_kernel_spmd

NCORES = 8
B, I, K, D, E = 64, 2048, 16, 8, 16
BL = B // NCORES          # 8 batches per core
NJ = I // 16              # 128 blocks of 16 input capsules
JC = 32                   # j-blocks per routing chunk
NCH = NJ // JC            # 4 chunks
WCH = 16                  # j-blocks per W DMA chunk
EPS = 1e-7
MAGIC = 0x5F3759DF

bf16 = mybir.dt.bfloat16
f32 = mybir.dt.float32
i32 = mybir.dt.int32
FT = mybir.ActivationFunctionType
ALU = mybir.AluOpType

TRACE = False
_NC_CACHE = {}


def _bc(ap, shape):
    try:
        return ap.broadcast_to(shape)
    except Exception:
        return ap.to_broadcast(shape)


def _capsule_kernel(tc, vout, ablk, absum, wmv, repmat):
    nc = tc.nc
    with (
        tc.tile_pool(name="singles", bufs=1) as singles,
        tc.tile_pool(name="wstream", bufs=2) as wpool,
        tc.tile_pool(name="crps", bufs=2, space="PSUM") as crps,
        tc.tile_pool(name="sps", bufs=1, space="PSUM") as sps,
        tc.tile_pool(name="saccps", bufs=1, space="PSUM") as saccps,
        tc.tile_pool(name="vrps", bufs=1, space="PSUM") as vrps,
        tc.tile_pool(name="bigchunk", bufs=1) as bchp,
        tc.tile_pool(name="chunk", bufs=2) as chp,
        tc.tile_pool(name="softk", bufs=4) as skp,
        tc.tile_pool(name="small", bufs=2) as small,
    ):
        repm = singles.tile([8, 128], bf16)
        nc.sync.dma_start(out=repm, in_=repmat)
        absum_sb = singles.tile([128, NJ, 8], bf16)
        nc.sync.dma_start(out=absum_sb, in_=absum)

        u_bf = singles.tile([128, NJ, E, K], bf16)      # 8 MiB, layout (j, e, k)
        a_r1 = singles.tile([128, NJ, K], bf16)         # agreement logits A(v0)
        L = singles.tile([128, NJ, 8, K], bf16)         # blockdiag C, (j, b', k)
        nc.gpsimd.memset(L, 0.0)                        # zeros persist; only diag rewritten

        ablk_sb = singles.tile([128, NJ, 128], bf16)

        # ---- phase 1: u_hat + s0 = (1/16) sum_i u_hat ----
        s0_ps = sps.tile([8, E, K], f32)
        g_idx = 0
        for c in range(NJ // WCH):
            if c < 4:
                nc.sync.dma_start(out=ablk_sb[:, 32 * c:32 * (c + 1)],
                                  in_=ablk[:, 32 * c:32 * (c + 1)])
            wt = wpool.tile([128, WCH, 256], bf16)
            nc.sync.dma_start(out=wt, in_=wmv[:, c * WCH:(c + 1) * WCH])
            for g in range(WCH // 4):
                j0 = c * WCH + g * 4
                ps = crps.tile([128, 4, 256], f32)
                for jj in range(4):
                    j = j0 + jj
                    nc.tensor.matmul(ps[:, jj], lhsT=ablk_sb[:, j],
                                     rhs=wt[:, g * 4 + jj],
                                     start=True, stop=True, skip_group_check=True)
                    nc.tensor.matmul(s0_ps, lhsT=absum_sb[:, j],
                                     rhs=wt[:, g * 4 + jj],
                                     start=(j == 0), stop=(j == NJ - 1),
                                     skip_group_check=True)
                dst = u_bf[:, j0:j0 + 4]
                m = g_idx % 4
                g_idx += 1
                if m in (0, 1):
                    nc.scalar.activation(dst, ps, func=FT.Copy)
                elif m == 2:
                    nc.vector.tensor_copy(dst, ps)
                else:
                    nc.gpsimd.tensor_copy(dst, ps)

        def squash(sdiag, final):
            # sdiag [8, E, K] f32; returns v_rep [128, E, K] bf16 (unless final)
            sq = small.tile([8, E, K], f32, tag="sq")
            nc.vector.tensor_mul(sq, sdiag, sdiag)
            t8 = small.tile([8, 8, K], f32, tag="sq8")
            nc.vector.tensor_add(t8, sq[:, 0:8], sq[:, 8:16])
            t4 = small.tile([8, 4, K], f32, tag="sq4")
            nc.vector.tensor_add(t4, t8[:, 0:4], t8[:, 4:8])
            t2 = small.tile([8, 2, K], f32, tag="sq2")
            nc.vector.tensor_add(t2, t4[:, 0:2], t4[:, 2:4])
            sn = small.tile([8, K], f32, tag="sn")
            nc.vector.tensor_add(sn, t2[:, 0], t2[:, 1])
            sne = small.tile([8, K], f32, tag="sne")
            nc.vector.tensor_scalar_add(sne, sn, EPS)
            # rsqrt(sne) via bit trick + 2 Newton steps, all on DVE
            y0i = small.tile([8, K], i32, tag="y0i")
            nc.vector.tensor_scalar(y0i, sne.bitcast(i32), 1, None,
                                    op0=ALU.logical_shift_right)
            y0 = small.tile([8, K], i32, tag="y0")
            nc.vector.tensor_scalar(y0, y0i, -1, MAGIC, op0=ALU.mult, op1=ALU.add)
            yc = y0.bitcast(f32)
            for step in range(2):
                t = small.tile([8, K], f32, tag=f"nt{step}")
                nc.vector.tensor_mul(t, sne, yc)
                t2n = small.tile([8, K], f32, tag=f"nt2{step}")
                nc.vector.tensor_mul(t2n, t, yc)
                h = small.tile([8, K], f32, tag=f"nh{step}")
                nc.vector.tensor_scalar(h, t2n, -0.5, 1.5, op0=ALU.mult, op1=ALU.add)
                yn = small.tile([8, K], f32, tag=f"ny{step}")
                nc.vector.tensor_mul(yn, yc, h)
                yc = yn
            onep = small.tile([8, K], f32, tag="onep")
            nc.vector.tensor_scalar_add(onep, sn, 1.0)
            rec = small.tile([8, K], f32, tag="rec")
            nc.vector.reciprocal(rec, onep)
            fac = small.tile([8, K], f32, tag="fac")
            nc.vector.tensor_mul(fac, sn, yc)
            fac2 = small.tile([8, K], f32, tag="fac2")
            nc.vector.tensor_mul(fac2, fac, rec)
            if final:
                vfin = small.tile([8, K, E], f32, tag="vfin")
                # vfin[b,k,e] = sdiag[b,e,k] * fac2[b,k]; strided out reorder
                nc.vector.tensor_mul(
                    vfin.rearrange("b k e -> b e k"), sdiag,
                    _bc(fac2.unsqueeze(1), [8, E, K]))
                return vfin
            vbf = small.tile([8, E, K], bf16, tag="vbf")
            nc.vector.tensor_mul(vbf, sdiag,
                                 _bc(fac2.unsqueeze(1), [8, E, K]))
            vr_ps = vrps.tile([128, E, K], f32)
            nc.tensor.matmul(vr_ps, lhsT=repm, rhs=vbf,
                             start=True, stop=True, skip_group_check=True)
            v_rep = small.tile([128, E, K], bf16, tag="vrep")
            nc.vector.tensor_copy(v_rep, vr_ps)
            return v_rep

        s0_sb = small.tile([8, E, K], f32, tag="s0sb")
        nc.vector.tensor_copy(s0_sb, s0_ps)
        v_rep = squash(s0_sb, False)

        # ---- routing iterations ----
        for r in (1, 2):
            s_ps = saccps.tile([128, E, K], f32)
            # pass A: agreement trees + exp (ACT overlaps next chunk's tree)
            exs = []
            for ci in range(NCH):
                jsl = slice(ci * JC, (ci + 1) * JC)
                prod = bchp.tile([128, JC, E, K], bf16, tag="prod")
                nc.vector.tensor_mul(prod, u_bf[:, jsl],
                                     _bc(v_rep.unsqueeze(1), [128, JC, E, K]))
                a8 = bchp.tile([128, JC, 8, K], bf16, tag="a8")
                nc.vector.tensor_add(a8, prod[:, :, 0:8], prod[:, :, 8:16])
                a4 = bchp.tile([128, JC, 4, K], bf16, tag="a4")
                nc.vector.tensor_add(a4, a8[:, :, 0:4], a8[:, :, 4:8])
                a2 = bchp.tile([128, JC, 2, K], bf16, tag="a2")
                nc.vector.tensor_add(a2, a4[:, :, 0:2], a4[:, :, 2:4])
                if r == 1:
                    nc.vector.tensor_add(a_r1[:, jsl], a2[:, :, 0], a2[:, :, 1])
                    ex_src = a_r1[:, jsl]
                else:
                    acomb = skp.tile([128, JC, K], bf16, tag="acomb")
                    nc.vector.tensor_add(acomb, a2[:, :, 0], a2[:, :, 1])
                    nc.vector.tensor_add(acomb, acomb, a_r1[:, jsl])
                    ex_src = acomb
                ex = skp.tile([128, JC, K], bf16, tag="ex")
                nc.scalar.activation(ex, ex_src, func=FT.Exp)
                exs.append(ex)
            # pass B: softmax-normalize, scatter into L, accumulate s on PE
            for ci in range(NCH):
                jsl = slice(ci * JC, (ci + 1) * JC)
                ex = exs[ci]
                k8 = chp.tile([128, JC, 8], bf16, tag="k8")
                nc.vector.tensor_add(k8, ex[:, :, 0:8], ex[:, :, 8:16])
                k4 = chp.tile([128, JC, 4], bf16, tag="k4")
                nc.vector.tensor_add(k4, k8[:, :, 0:4], k8[:, :, 4:8])
                k2 = chp.tile([128, JC, 2], bf16, tag="k2")
                nc.vector.tensor_add(k2, k4[:, :, 0:2], k4[:, :, 2:4])
                ks = chp.tile([128, JC], f32, tag="ks")
                nc.vector.tensor_add(ks, k2[:, :, 0], k2[:, :, 1])
                krec = chp.tile([128, JC], f32, tag="krec")
                nc.vector.reciprocal(krec, ks)
                cch = chp.tile([128, JC, K], bf16, tag="cch")
                nc.vector.tensor_mul(cch, ex, _bc(krec.unsqueeze(2), [128, JC, K]))
                for b in range(8):
                    eng = nc.vector if b % 2 == 0 else nc.gpsimd
                    eng.tensor_copy(L[b * 16:(b + 1) * 16, jsl, b, :],
                                    cch[b * 16:(b + 1) * 16])
                for jj in range(JC):
                    j = ci * JC + jj
                    nc.tensor.matmul(s_ps, lhsT=L[:, j], rhs=u_bf[:, j],
                                     start=(j == 0), stop=(j == NJ - 1),
                                     skip_group_check=True)
            sdiag = small.tile([8, E, K], f32, tag="sdiag")
            for k in range(K):
                nc.gpsimd.tensor_copy(sdiag[:, :, k], s_ps[k::16, :, k])
            if r == 2:
                vfin = squash(sdiag, True)
                nc.sync.dma_start(out=vout, in_=vfin)
            else:
                v_rep = squash(sdiag, False)


def _build():
    if "nc" in _NC_CACHE:
        return _NC_CACHE["nc"]
    nc = bacc.Bacc("TRN2", target_bir_lowering=False, debug=False,
                   num_devices=NCORES)
    ablk = nc.dram_tensor("ablk", [128, NJ, 128], bf16, kind="ExternalInput").ap()
    absum = nc.dram_tensor("absum", [128, NJ, 8], bf16, kind="ExternalInput").ap()
    wmv = nc.dram_tensor("wmv", [128, NJ, 256], bf16, kind="ExternalInput").ap()
    repmat = nc.dram_tensor("repmat", [8, 128], bf16, kind="ExternalInput").ap()
    vout = nc.dram_tensor("vout", [BL, K, E], f32, kind="ExternalOutput").ap()
    with tile.TileContext(nc) as tc:
        _capsule_kernel(tc, vout, ablk, absum, wmv, repmat)
    nc.compile()
    _NC_CACHE["nc"] = nc
    return nc


def kernel(inputs, W):
    inputs = np.asarray(inputs, np.float32)
    W = np.asarray(W, np.float32)
    nc = _build()

    # W[i,k,d,e] -> [j, iu, d, e, k] -> [(iu d)=128, j, (e k)=256] bf16
    Wb = np.ascontiguousarray(
        W.reshape(NJ, 16, K, D, E).transpose(0, 1, 3, 4, 2)
        .reshape(NJ, 128, 256).transpose(1, 0, 2)
    ).astype(ml_dtypes.bfloat16)

    repmat_np = np.zeros((8, 128), np.float32)
    repmat_np[np.arange(128) // 16, np.arange(128)] = 1.0
    repmat_np = repmat_np.astype(ml_dtypes.bfloat16)

    in_maps = []
    for c in range(NCORES):
        inp_c = inputs[c * BL:(c + 1) * BL]               # [8, 2048, 8]
        inp_t = inp_c.reshape(BL, NJ, 16, D)              # b, j, iu, d
        ab = np.zeros((16, D, NJ, BL, 16), np.float32)    # iu d j b iu2
        for iu in range(16):
            ab[iu, :, :, :, iu] = inp_t[:, :, iu, :].transpose(2, 1, 0)
        ab = ab.reshape(128, NJ, 128).astype(ml_dtypes.bfloat16)
        # absum[(iu,d), j, b] = x[b, 16j+iu, d] / 16  (dense; for s0 off W tiles)
        asm = (inp_t.transpose(2, 3, 1, 0) / 16.0)        # iu d j b
        asm = asm.reshape(128, NJ, 8).astype(ml_dtypes.bfloat16)
        in_maps.append({"ablk": ab, "absum": asm, "wmv": Wb,
                        "repmat": repmat_np})

    br = run_bass_kernel_spmd(nc, in_maps, core_ids=list(range(NCORES)),
                              trace=TRACE)
    if br.exec_time_ns is not None:
        print(f"HW exec time: {br.exec_time_ns} ns")
    out = np.concatenate([r["vout"] for r in br.results], axis=0)
    return out.astype(np.float32)
